# revision 1
# baseline (speedup 1.0000x reference)
"""SoftDTW loss (AbstractDTW, gamma=1) Trainium2 Bass kernel.

Algorithm: exp-space linearization of the SoftDTW DP. With E = exp(-R) and
W = exp(-D), the recurrence R[i,j] = D[i,j] + softmin(R[i-1,j-1], R[i-1,j],
R[i,j-1]) becomes linear:

    E[i,j] = W[i,j] * (E[i-1,j-1] + E[i-1,j] + E[i,j-1])

Per row this is a first-order linear scan along j, which maps onto the DVE
`tensor_tensor_scan` instruction with op0=add, op1=mult:

    state = (S[j] + state) * W[j],   S[j] = E[i-1,j-1] + E[i-1,j]

Parallelization per core: 8 batch items; the 1024 columns are split into
K=16 blocks of C=64. SBUF partition p = k*8 + b (128 partitions). Wavefront
skew: slot k processes row i at step t = i + 2k, so cross-slot boundary
values (last column of slot k-1) arrive 2 steps ahead of use. The boundary
column is routed across partitions with a tiny PE matmul (shift-by-8
matrix) and scale-corrected on the Scalar engine.

Numerical range: per-slot scales, every factor an exact power of 2 built by
exponent-field bit arithmetic on DVE (HW Ln/Exp LUTs are unsafe in the far
tails). Scales re-anchor every Q steps to max(own block peak, left
neighbor's candidate) minus 64 (headroom so the boundary column -- the
corridor's leading edge -- stays unflushed). The final CxC block is
recomputed on host in float64 log space from exported boundary values,
since the DP corner can legitimately sit beyond f32 range below the block
peak. Batch dim (64) is sharded 8 ways across cores; mean taken on host.
"""

import os
import sys
from contextlib import ExitStack

import numpy as np

sys.path.insert(0, "/opt/trn_rl_repo")

import concourse.bass as bass  # noqa: E402
import concourse.tile as tile  # noqa: E402
from concourse import bacc, mybir  # noqa: E402
from concourse import bass_utils  # noqa: E402

AF = mybir.ActivationFunctionType
ALU = mybir.AluOpType

NCORES = 8
B = 8          # batch per core
K = 16         # column slots
N = 1024       # sequence length
C = N // K     # columns per slot
P = K * B      # 128 partitions
NSTEP = N + 2 * (K - 1)          # 1054 wavefront steps
RBLK = 64                        # W-production chunk (steps)
NCHUNK = (NSTEP + RBLK - 1) // RBLK
SXLEN = NCHUNK * RBLK            # staggered snake buffer length (1088)
Q = 8                            # rescale cadence
RESCALE_STEPS = [t for t in range(1, NSTEP) if t % Q == 0]
NRESC = len(RESCALE_STEPS)
PAD = np.float32(1e4)            # out-of-range snake pad -> W == 0
TAIL0 = ((N - C - 1 + 2 * (K - 2)) // Q) * Q   # export window start step
NTAIL = NSTEP - TAIL0                          # boundary-column exports
EROW_STEP = N - C - 1 + 2 * (K - 1)            # step producing row N-C-1
NHIST = (NSTEP - 1 - TAIL0) // Q + 1           # lacc snapshots in window


def build_bass():
    """Build the per-core Bass program (SPMD: same program on all cores)."""
    nc = bacc.Bacc(
        "TRN2",
        target_bir_lowering=False,
        debug=False,
        enable_asserts=False,
        num_devices=NCORES,
    )
    f32 = mybir.dt.float32
    cx_d = nc.dram_tensor("cx", [P, C], f32, kind="ExternalInput").ap()
    cy_d = nc.dram_tensor("cy", [P, C], f32, kind="ExternalInput").ap()
    sx_d = nc.dram_tensor("sx", [P, SXLEN], f32, kind="ExternalInput").ap()
    sy_d = nc.dram_tensor("sy", [P, SXLEN], f32, kind="ExternalInput").ap()
    sh_d = nc.dram_tensor("shift", [P, P], f32, kind="ExternalInput").ap()
    se_d = nc.dram_tensor("sel", [P, P], f32, kind="ExternalInput").ap()
    out_d = nc.dram_tensor("out", [P, 2], f32, kind="ExternalOutput").ap()
    bcol_d = nc.dram_tensor("bcol", [P, NTAIL], f32, kind="ExternalOutput").ap()
    erow_d = nc.dram_tensor("erow", [P, C + 1], f32, kind="ExternalOutput").ap()
    lh_d = nc.dram_tensor("lh", [P, NHIST], f32, kind="ExternalOutput").ap()

    with TileKernel(nc) as tk:
        tk.body(cx_d, cy_d, sx_d, sy_d, sh_d, se_d, out_d, bcol_d, erow_d, lh_d)
    nc.compile()
    return nc


class TileKernel:
    def __init__(self, nc):
        self.nc = nc
        self.ctx = ExitStack()
        self.tc = tile.TileContext(nc)

    def __enter__(self):
        self.ctx.__enter__()
        self.tc.__enter__()
        return self

    def __exit__(self, *a):
        self.ctx.__exit__(*a)  # close tile pools before scheduling
        return self.tc.__exit__(*a)

    def body(self, cx_d, cy_d, sx_d, sy_d, sh_d, se_d, out_d, bcol_d, erow_d, lh_d):
        nc = self.nc
        tc = self.tc
        ctx = self.ctx
        f32 = mybir.dt.float32

        const = ctx.enter_context(tc.tile_pool(name="const", bufs=1))
        wpool = ctx.enter_context(tc.tile_pool(name="wp", bufs=2))
        dpool = ctx.enter_context(tc.tile_pool(name="dp", bufs=2))
        epool = ctx.enter_context(tc.tile_pool(name="ep", bufs=6))
        spool = ctx.enter_context(tc.tile_pool(name="sp", bufs=3))
        small = ctx.enter_context(tc.tile_pool(name="sm", bufs=2))
        psum = ctx.enter_context(tc.tile_pool(name="ps", bufs=2, space="PSUM"))

        # ---- constants / inputs
        cx = const.tile([P, C], f32)
        cy = const.tile([P, C], f32)
        sx = const.tile([P, SXLEN], f32)
        sy = const.tile([P, SXLEN], f32)
        shm = const.tile([P, P], f32)
        sel = const.tile([P, P], f32)
        nc.sync.dma_start(sel[:], se_d)
        nc.sync.dma_start(cx[:], cx_d)
        nc.sync.dma_start(cy[:], cy_d)
        nc.sync.dma_start(sx[:], sx_d)
        nc.sync.dma_start(sy[:], sy_d)
        nc.sync.dma_start(shm[:], sh_d)

        # accumulated base-2 exponent per partition (exact f32 integers)
        lacc_a = const.tile([P, 1], f32)
        lacc_b = const.tile([P, 1], f32)
        nc.vector.memset(lacc_a[:], 0.0)
        lacc_cur, lacc_alt = lacc_a, lacc_b
        # r = 2^(Lacc[p-8]-Lacc[p]) as bitcast-f32 (exact power of 2)
        r_t = const.tile([P, 1], mybir.dt.uint32)
        nc.vector.memset(r_t[:].bitcast(f32), 1.0)
        # additive mask: slot 0 rows get a huge negative so max() keeps own
        negmask = const.tile([P, 1], f32)
        nc.vector.memset(negmask[:], 0.0)
        nc.vector.memset(negmask[0:B, :], -3.0e8)

        bcol = const.tile([P, NTAIL], f32)
        erow = const.tile([P, C + 1], f32)
        lh = const.tile([P, NHIST], f32)

        # E tile ring; e[t] has C+1 cols: col 0 = boundary, 1..C = block
        e_tiles = {}
        e_init = epool.tile([P, C + 1], f32)
        nc.vector.memset(e_init[:], 0.0)
        nc.vector.memset(e_init[0:B, 0:1], 1.0)   # DP seed delta (slot 0)
        e_tiles[-1] = e_init

        w_cur = None
        for t in range(NSTEP):
            # ---- bulk W production for steps [t, t+RBLK)
            if t % RBLK == 0:
                dx = dpool.tile([P, RBLK, C], f32, tag="dx")
                dy = dpool.tile([P, RBLK, C], f32, tag="dy")
                w_cur = wpool.tile([P, RBLK, C], f32, tag="w")
                cxb = cx[:].unsqueeze(1).broadcast_to([P, RBLK, C])
                cyb = cy[:].unsqueeze(1).broadcast_to([P, RBLK, C])
                sxb = sx[:, t:t + RBLK].unsqueeze(2).broadcast_to([P, RBLK, C])
                syb = sy[:, t:t + RBLK].unsqueeze(2).broadcast_to([P, RBLK, C])
                nc.vector.tensor_sub(dx[:], cxb, sxb)
                nc.vector.tensor_sub(dy[:], cyb, syb)
                nc.scalar.activation(dx[:], dx[:], AF.Square)
                nc.scalar.activation(dy[:], dy[:], AF.Square)
                nc.vector.tensor_add(dx[:], dx[:], dy[:])
                nc.scalar.activation(w_cur[:], dx[:], AF.Exp, scale=-1.0)

            # ---- rescale epoch boundary.
            # Per-slot scales, all exact powers of 2 (integer exponents in
            # f32; no Ln/Exp on device -- HW Ln is inaccurate below ~1e-20).
            # New scale: Lnew[p] = max(Lacc[p]+E[p], Lacc[p-8]+E[p-8]) where
            # E = floor(log2(row max)). The neighbor term adopts the left
            # slot's scale whenever its mass dominates (corridor handoff),
            # so the boundary ratio r = 2^(Lnew[p-8]-Lnew[p]) stays in f32
            # range whenever the routed mass is relevant.
            if t % Q == 0 and t > 0:
                prev = e_tiles[t - 1]
                m = small.tile([P, 1], f32, tag="m")
                z = small.tile([P, 1], f32, tag="z")
                mz = small.tile([P, 1], f32, tag="mz")
                shf = small.tile([P, 1], f32, tag="shf")
                lcand = small.tile([P, 1], f32, tag="lcand")
                nb = small.tile([P, 1], f32, tag="nb")
                sfe = small.tile([P, 1], f32, tag="sfe")
                sfu = small.tile([P, 1], mybir.dt.uint32, tag="sfu")
                sfu2 = small.tile([P, 1], mybir.dt.uint32, tag="sfu2")
                dl = small.tile([P, 1], f32, tag="dl")
                lnew = lacc_alt

                nc.vector.tensor_reduce(
                    m[:], prev[:], axis=mybir.AxisListType.X, op=ALU.max
                )
                # own-scale candidate: Lacc + E - 64, E = exponent(m).
                # Stale slots (m~0) get filler 2^64 so their E-64 term is 0.
                nc.vector.tensor_scalar(
                    z[:], m[:], 1e-37, 1.8446744e19, op0=ALU.is_le, op1=ALU.mult
                )
                nc.vector.tensor_add(mz[:], m[:], z[:])
                nc.vector.tensor_scalar(
                    sfu[:], mz[:].bitcast(mybir.dt.uint32), 23, None,
                    op0=ALU.logical_shift_right,
                )
                nc.vector.tensor_copy(shf[:], sfu[:])          # u32 -> f32
                # E - 64: anchor the scale 2^64 below the block peak so the
                # boundary column (corridor leading edge) stays unflushed
                nc.vector.tensor_scalar(shf[:], shf[:], -191.0, None, op0=ALU.add)
                nc.vector.tensor_add(lcand[:], lacc_cur[:], shf[:])
                # neighbor candidate via PE shift; slot 0 forced to keep own.
                # A slot with no mass (z=1) must not export its (stale)
                # scale, so its shifted candidate is pushed to -inf-ish.
                lsrc = small.tile([P, 1], f32, tag="lsrc")
                nc.vector.tensor_scalar(lsrc[:], z[:], -3.0e8, None, op0=ALU.mult)
                nc.vector.tensor_add(lsrc[:], lsrc[:], lcand[:])
                psl = psum.tile([P, 1], f32, tag="psl")
                nc.tensor.matmul(psl[:], shm[:], lsrc[:], start=True, stop=True)
                nc.vector.tensor_tensor(nb[:], psl[:], negmask[:], op=ALU.add)
                # lnew = lcand + v*(max(lsrc, nb) - lcand), v = neighbor valid
                mx = small.tile([P, 1], f32, tag="mx")
                vv = small.tile([P, 1], f32, tag="vv")
                nc.vector.tensor_tensor(mx[:], lsrc[:], nb[:], op=ALU.max)
                nc.vector.tensor_scalar(vv[:], nb[:], -1e8, None, op0=ALU.is_ge)
                nc.vector.tensor_tensor(mx[:], mx[:], lcand[:], op=ALU.subtract)
                nc.vector.tensor_tensor(mx[:], mx[:], vv[:], op=ALU.mult)
                nc.vector.tensor_add(lnew[:], lcand[:], mx[:])
                # value scale factor 2^(Lacc-Lnew), clamped to +-126
                nc.vector.tensor_tensor(
                    sfe[:], lacc_cur[:], lnew[:], op=ALU.subtract
                )
                nc.vector.tensor_scalar(
                    sfe[:], sfe[:], -126.0, 126.0, op0=ALU.max, op1=ALU.min
                )
                nc.vector.tensor_scalar(sfe[:], sfe[:], 127.0, None, op0=ALU.add)
                nc.vector.tensor_copy(sfu2[:], sfe[:])         # f32 -> u32
                nc.vector.tensor_scalar(
                    sfu2[:], sfu2[:], 23, None, op0=ALU.logical_shift_left
                )
                sf_ap = sfu2[:].bitcast(f32)
                nc.vector.tensor_scalar_mul(prev[:], prev[:], sf_ap)
                nc.vector.tensor_scalar_mul(
                    e_tiles[t][:, 0:1], e_tiles[t][:, 0:1], sf_ap
                )
                nc.vector.tensor_scalar_mul(
                    e_tiles[t + 1][:, 0:1], e_tiles[t + 1][:, 0:1], sf_ap
                )
                # r = 2^(Lnew[p-8]-Lnew[p]), clamped; slot 0 -> 2^-126 ~ 0
                psl2 = psum.tile([P, 1], f32, tag="psl2")
                nc.tensor.matmul(psl2[:], shm[:], lnew[:], start=True, stop=True)
                nc.vector.tensor_tensor(dl[:], psl2[:], negmask[:], op=ALU.add)
                nc.vector.tensor_tensor(dl[:], dl[:], lnew[:], op=ALU.subtract)
                nc.vector.tensor_scalar(
                    dl[:], dl[:], -126.0, 110.0, op0=ALU.max, op1=ALU.min
                )
                nc.vector.tensor_scalar(dl[:], dl[:], 127.0, None, op0=ALU.add)
                nc.vector.tensor_copy(r_t[:], dl[:])           # f32 -> u32
                nc.vector.tensor_scalar(
                    r_t[:], r_t[:], 23, None, op0=ALU.logical_shift_left
                )
                if t >= TAIL0:
                    nc.vector.tensor_copy(
                        lh[:, (t - TAIL0) // Q:(t - TAIL0) // Q + 1], lnew[:]
                    )
                lacc_cur, lacc_alt = lnew, lacc_cur

            # ---- wavefront step t
            eprev = e_tiles[t - 1]
            if t not in e_tiles:  # t = 0, 1: no boundary was routed yet
                ecur = epool.tile([P, C + 1], f32)
                nc.vector.memset(ecur[:, 0:1], 0.0)
                e_tiles[t] = ecur
            ecur = e_tiles[t]
            s_t = spool.tile([P, C], f32)
            nc.vector.tensor_add(s_t[:], eprev[:, 0:C], eprev[:, 1:C + 1])
            wsl = w_cur[:, t % RBLK, :]
            nc.vector.tensor_tensor_scan(
                ecur[:, 1:C + 1], s_t[:], wsl, ecur[:, 0:1],
                op0=ALU.add, op1=ALU.mult,
            )
            if t >= TAIL0:
                nc.vector.tensor_copy(
                    bcol[:, t - TAIL0:t - TAIL0 + 1], ecur[:, C:C + 1]
                )
            if t == EROW_STEP:
                nc.vector.tensor_copy(erow[:], ecur[:])
            # route last col to slot k+1 (partition +8) for step t+2
            if t + 2 < NSTEP:
                ps = psum.tile([P, 1], f32, tag="sh")
                nc.tensor.matmul(
                    ps[:], shm[:], ecur[:, C:C + 1], start=True, stop=True
                )
                enx = epool.tile([P, C + 1], f32)
                e_tiles[t + 2] = enx
                nc.scalar.activation(
                    enx[:, 0:1], ps[:], AF.Copy, bias=0.0,
                    scale=r_t[:].bitcast(f32),
                )
            e_tiles.pop(t - 2, None)

        # ---- finalization: out = [E_last, Lacc]
        outt = const.tile([P, 2], f32)
        nc.vector.tensor_copy(outt[:, 0:1], e_tiles[NSTEP - 1][:, C:C + 1])
        nc.vector.tensor_copy(outt[:, 1:2], lacc_cur[:])
        nc.sync.dma_start(out_d, outt[:])
        nc.sync.dma_start(bcol_d, bcol[:])
        nc.sync.dma_start(erow_d, erow[:])
        nc.sync.dma_start(lh_d, lh[:])


def prep_core_inputs(snake, contour):
    """snake, contour: [B, N, 2] float32 -> input dict for one core."""
    cx = np.empty((P, C), np.float32)
    cy = np.empty((P, C), np.float32)
    sx = np.full((P, SXLEN), PAD, np.float32)
    sy = np.full((P, SXLEN), PAD, np.float32)
    for k in range(K):
        for b in range(B):
            p = k * B + b
            cx[p] = contour[b, k * C:(k + 1) * C, 0]
            cy[p] = contour[b, k * C:(k + 1) * C, 1]
            lo = 2 * k
            sx[p, lo:lo + N] = snake[b, :, 0]
            sy[p, lo:lo + N] = snake[b, :, 1]
    shift = np.zeros((P, P), np.float32)
    for q in range(P - B):
        shift[q, q + B] = 1.0
    sel = np.zeros((P, P), np.float32)
    for q in range(P):
        for p in range(P):
            if q % B == p % B:
                sel[q, p] = 1.0
    return {"cx": cx, "cy": cy, "sx": sx, "sy": sy, "shift": shift, "sel": sel}


_CACHED = {}


def _get_nc():
    if "nc" not in _CACHED:
        _CACHED["nc"] = build_bass()
    return _CACHED["nc"]


def host_finish(out_map, snake, contour):
    """Recompute the final CxC block in float64 log space from exported
    boundaries (the corner can sit beyond f32 range below the block peak).
    snake, contour: [B, N, 2] for this core. Returns R[B]."""
    LN2 = np.log(2.0)
    bcol = out_map["bcol"].astype(np.float64)   # [P, NTAIL]
    erow = out_map["erow"].astype(np.float64)   # [P, C+1]
    lh = out_map["lh"].astype(np.float64)       # [P, NHIST]
    i0 = N - C
    res = np.empty(B)
    for b in range(B):
        p15 = (K - 1) * B + b
        p14 = (K - 2) * B + b
        with np.errstate(divide="ignore"):
            # R[i0-1, j], j = i0-1 .. N-1 (erow col 0 is j = i0-1)
            sc15 = lh[p15, (EROW_STEP - TAIL0) // Q]
            Rrow = -(np.log(erow[p15]) + LN2 * sc15)
            # R[i, i0-1], i = i0 .. N-1: slot-14 last col at step i + 2(K-2)
            tt = i0 + np.arange(C) + 2 * (K - 2)
            sc = lh[p14, (tt - TAIL0) // Q]
            Rcol = -(np.log(bcol[p14, tt - TAIL0]) + LN2 * sc)
        D = ((snake[b, i0:, None, :].astype(np.float64)
              - contour[b, None, i0:, :].astype(np.float64)) ** 2).sum(-1)
        Rm = np.full((C + 1, C + 1), np.inf)
        Rm[0, :] = Rrow
        Rm[1:, 0] = Rcol
        for ii in range(1, C + 1):
            dvals = D[ii - 1]
            rowm1 = Rm[ii - 1]
            rowc = Rm[ii]
            for jj in range(1, C + 1):
                v0, v1, v2 = rowm1[jj - 1], rowm1[jj], rowc[jj - 1]
                mn = min(v0, v1, v2)
                if mn == np.inf:
                    continue
                rowc[jj] = dvals[jj - 1] + mn - np.log(
                    np.exp(mn - v0) + np.exp(mn - v1) + np.exp(mn - v2)
                )
        res[b] = Rm[C, C]
    return res


def run(snake, contour, trace=False):
    """Returns (loss, results_obj)."""
    snake = np.asarray(snake, np.float32)
    contour = np.asarray(contour, np.float32)
    nbatch = snake.shape[0]
    assert nbatch == NCORES * B, (snake.shape, contour.shape)
    in_maps = [
        prep_core_inputs(
            snake[c * B:(c + 1) * B], contour[c * B:(c + 1) * B]
        )
        for c in range(NCORES)
    ]
    nc = _get_nc()
    res = bass_utils.run_bass_kernel_spmd(
        nc, in_maps, core_ids=list(range(NCORES)), trace=trace
    )
    rs = []
    for c in range(NCORES):
        rs.append(host_finish(
            res.results[c],
            snake[c * B:(c + 1) * B], contour[c * B:(c + 1) * B],
        ))
    loss = np.mean(np.concatenate(rs), dtype=np.float64)
    return np.float32(loss), res


def kernel(snake, contour):
    loss, _ = run(snake, contour, trace=False)
    return np.array(loss, dtype=np.float32)



# revision 6
# speedup vs baseline: 2.1333x; 2.1333x over previous
"""SoftDTW loss (AbstractDTW, gamma=1) Trainium2 Bass kernel.

Algorithm: exp-space linearization of the SoftDTW DP. With E = exp(-R) and
W = exp(-D), the recurrence R[i,j] = D[i,j] + softmin(R[i-1,j-1], R[i-1,j],
R[i,j-1]) becomes linear:

    E[i,j] = W[i,j] * (E[i-1,j-1] + E[i-1,j] + E[i,j-1])

Per row this is a first-order linear scan along j, which maps onto the DVE
`tensor_tensor_scan` instruction with op0=add, op1=mult:

    state = (S[j] + state) * W[j],   S[j] = E[i-1,j-1] + E[i-1,j]

Parallelization per core: 8 batch items; the 1024 columns are split into
K=16 blocks of C=64. SBUF partition p = k*8 + b (128 partitions). Wavefront
skew: slot k processes row i at step t = i + 2k, so cross-slot boundary
values (last column of slot k-1) arrive 2 steps ahead of use.

Engine split (the DVE add->scan chain is the critical path; every DVE op
larger than the ~95ns RAW-semaphore gap pushes the whole schedule):
  - DVE: per-step pair-sum + scan + boundary import (PSUM->SBUF with the
    power-of-two scale ratio r, hidden in the RAW gap), plus the rescale
    chain as tiny [P,1] ops spread into chain gaps.
  - Pool (gpsimd): all W-production tensor passes (sub, sub, add) in
    [P,16,C] slices. Nothing latency-critical lives here.
  - Act: W-production activations (Square x2, Exp).
  - PE: boundary route (shift-by-8 matmul) + 2 rescale routes per epoch.

Numerical range: per-slot scales, every factor an exact power of 2 built by
exponent-field bit arithmetic (HW Ln/Exp LUTs are unsafe in the far tails).
Scales re-anchor every Q=16 steps; the anchor row is sampled DLEAD=8 steps
early so the whole scale computation runs off the critical chain, and only
one tensor_scalar_mul (plus two [P,1] boundary fixups) lands on it per
epoch. The final CxC block is recomputed on host in float64 log space from
exported boundary values. Batch dim (64) sharded 8 ways; mean on host.
"""

import sys
from contextlib import ExitStack

import numpy as np

sys.path.insert(0, "/opt/trn_rl_repo")

import concourse.bass as bass  # noqa: E402
import concourse.tile as tile  # noqa: E402
from concourse import bacc, mybir  # noqa: E402
from concourse import bass_utils  # noqa: E402

AF = mybir.ActivationFunctionType
ALU = mybir.AluOpType

NCORES = 8
B = 8          # batch per core
K = 16         # column slots
N = 1024       # sequence length
C = N // K     # columns per slot
P = K * B      # 128 partitions
NSTEP = N + 2 * (K - 1)          # 1054 wavefront steps
RBLK = 64                        # W-production chunk (steps)
QTR = 16                         # production slice (steps)
NCHUNK = (NSTEP + RBLK - 1) // RBLK
SXLEN = NCHUNK * RBLK            # staggered snake buffer length (1088)
Q = 16                           # rescale cadence
DLEAD = 8                        # rescale anchor staleness (steps)
RESCALE_STEPS = [t for t in range(Q, NSTEP) if t % Q == 0]
PAD = np.float32(1e4)            # out-of-range snake pad -> W == 0
TAIL0 = ((N - C - 1 + 2 * (K - 2)) // Q) * Q   # export window start step
NTAIL = NSTEP - TAIL0                          # boundary-column exports
EROW_STEP = N - C - 1 + 2 * (K - 1)            # step producing row N-C-1
NHIST = (NSTEP - 1 - TAIL0) // Q + 1           # lacc snapshots in window
HEADROOM = -191.0                # -(127 + 64): anchor 2^64 below block peak
TWO64 = 1.8446744e19             # 2^64 filler so empty slots keep lacc


def build_bass():
    """Build the per-core Bass program (SPMD: same program on all cores)."""
    nc = bacc.Bacc(
        "TRN2",
        target_bir_lowering=False,
        debug=False,
        enable_asserts=False,
        num_devices=NCORES,
    )
    f32 = mybir.dt.float32
    cx_d = nc.dram_tensor("cx", [P, C], f32, kind="ExternalInput").ap()
    cy_d = nc.dram_tensor("cy", [P, C], f32, kind="ExternalInput").ap()
    sx_d = nc.dram_tensor("sx", [P, SXLEN], f32, kind="ExternalInput").ap()
    sy_d = nc.dram_tensor("sy", [P, SXLEN], f32, kind="ExternalInput").ap()
    sh_d = nc.dram_tensor("shift", [P, P], f32, kind="ExternalInput").ap()
    out_d = nc.dram_tensor("out", [P, 2], f32, kind="ExternalOutput").ap()
    bcol_d = nc.dram_tensor("bcol", [P, NTAIL], f32, kind="ExternalOutput").ap()
    erow_d = nc.dram_tensor("erow", [P, C + 1], f32, kind="ExternalOutput").ap()
    lh_d = nc.dram_tensor("lh", [P, NHIST], f32, kind="ExternalOutput").ap()

    with TileKernel(nc) as tk:
        tk.body(cx_d, cy_d, sx_d, sy_d, sh_d, out_d, bcol_d, erow_d, lh_d)
    nc.compile()
    return nc


class TileKernel:
    def __init__(self, nc):
        self.nc = nc
        self.ctx = ExitStack()
        self.tc = tile.TileContext(nc)

    def __enter__(self):
        self.ctx.__enter__()
        self.tc.__enter__()
        return self

    def __exit__(self, *a):
        self.ctx.__exit__(*a)  # close tile pools before scheduling
        return self.tc.__exit__(*a)

    def body(self, cx_d, cy_d, sx_d, sy_d, sh_d, out_d, bcol_d, erow_d, lh_d):
        nc = self.nc
        tc = self.tc
        ctx = self.ctx
        f32 = mybir.dt.float32
        u32 = mybir.dt.uint32

        const = ctx.enter_context(tc.tile_pool(name="const", bufs=1))
        wpool = ctx.enter_context(tc.tile_pool(name="wp", bufs=2))
        qpool = ctx.enter_context(tc.tile_pool(name="qp", bufs=2))
        epool = ctx.enter_context(tc.tile_pool(name="ep", bufs=6))
        spool = ctx.enter_context(tc.tile_pool(name="sp", bufs=3))
        small = ctx.enter_context(tc.tile_pool(name="sm", bufs=2))
        psum = ctx.enter_context(tc.tile_pool(name="ps", bufs=2, space="PSUM"))

        # ---- constants / inputs
        cx = const.tile([P, C], f32)
        cy = const.tile([P, C], f32)
        sx = const.tile([P, SXLEN], f32)
        sy = const.tile([P, SXLEN], f32)
        shm = const.tile([P, P], f32)
        nc.sync.dma_start(cx[:], cx_d)
        nc.sync.dma_start(cy[:], cy_d)
        nc.sync.dma_start(sx[:], sx_d)
        nc.sync.dma_start(sy[:], sy_d)
        nc.sync.dma_start(shm[:], sh_d)

        # accumulated base-2 exponent per partition (exact f32 integers)
        lacc_a = const.tile([P, 1], f32)
        lacc_b = const.tile([P, 1], f32)
        nc.vector.memset(lacc_a[:], 0.0)
        self.lacc_cur, self.lacc_alt = lacc_a, lacc_b
        # r per scale regime: 2^(L[p-8]-L[p]) as bitcast-f32 (power of 2)
        r_a = const.tile([P, 1], u32)
        r_b = const.tile([P, 1], u32)
        nc.vector.memset(r_a[:].bitcast(f32), 1.0)
        self.r_tiles = [r_a, r_b]
        # per-epoch value scale factor 2^(Lacc-Lnew) bits
        self.sf_t = const.tile([P, 1], u32)
        # additive mask: slot 0 rows get a huge negative so max() keeps own
        negmask = const.tile([P, 1], f32)
        nc.vector.memset(negmask[:], 0.0)
        nc.vector.memset(negmask[0:B, :], -3.0e8)

        bcol = const.tile([P, NTAIL], f32)
        erow = const.tile([P, C + 1], f32)
        lh = const.tile([P, NHIST], f32)

        self.nc_ = nc
        self.cx, self.cy, self.sx, self.sy = cx, cy, sx, sy
        self.shm, self.negmask = shm, negmask
        self.qpool, self.wpool, self.psum, self.small = qpool, wpool, psum, small
        self.lh, self.f32, self.u32 = lh, f32, u32

        # ---- prologue: produce W chunk 0 on DVE+Act (chain not started yet)
        w_tiles = {}
        w_tiles[0] = self.produce_chunk(0, nc.vector)

        # E tile ring; e[t] has C+1 cols: col 0 = boundary, 1..C = block
        e_tiles = {}
        e_init = epool.tile([P, C + 1], f32)
        nc.vector.memset(e_init[:], 0.0)
        nc.vector.memset(e_init[0:B, 0:1], 1.0)   # DP seed delta (slot 0)
        e_tiles[-1] = e_init

        self.rst = {}  # live rescale-chain state
        mm_ps = {}     # step -> psum tile of its boundary route

        for t in range(NSTEP):
            cchunk = t // RBLK
            off = t % RBLK
            # ---- W production for chunk c+1 on Pool/Act, sliced
            if cchunk + 1 < NCHUNK:
                self.produce_slice(cchunk + 1, off, w_tiles)

            # ---- rescale chain (tiny DVE ops in chain gaps + PE routes)
            Tnext = ((t + DLEAD) // Q) * Q
            if Tnext in RESCALE_STEPS and t >= Tnext - DLEAD:
                self.rescale_phase(t - (Tnext - DLEAD), Tnext, e_tiles)

            # ---- epoch boundary: apply scale to prev row
            is_epoch = t in RESCALE_STEPS
            eprev = e_tiles[t - 1]
            if is_epoch:
                sf_ap = self.sf_t[:].bitcast(f32)
                nc.vector.tensor_scalar_mul(eprev[:], eprev[:], sf_ap)

            # ---- wavefront step t
            if t not in e_tiles:  # t = 0, 1: no boundary was routed yet
                ecur = epool.tile([P, C + 1], f32)
                nc.vector.memset(ecur[:, 0:1], 0.0)
                e_tiles[t] = ecur
            ecur = e_tiles[t]
            s_t = spool.tile([P, C], f32)
            nc.vector.tensor_add(s_t[:], eprev[:, 0:C], eprev[:, 1:C + 1])
            # boundary import for step t+1 (fills the add->scan RAW gap)
            if t >= 1 and t + 1 < NSTEP:
                regime = (t - 1) // Q
                r_ap = self.r_tiles[regime % 2][:].bitcast(f32)
                enx = epool.tile([P, C + 1], f32)
                e_tiles[t + 1] = enx
                nc.vector.tensor_scalar_mul(enx[:, 0:1], mm_ps[t - 1][:], r_ap)
            if is_epoch:
                sf_ap = self.sf_t[:].bitcast(f32)
                nc.vector.tensor_scalar_mul(ecur[:, 0:1], ecur[:, 0:1], sf_ap)
                if t + 1 in e_tiles:
                    nc.vector.tensor_scalar_mul(
                        e_tiles[t + 1][:, 0:1], e_tiles[t + 1][:, 0:1], sf_ap
                    )
            wsl = w_tiles[cchunk][:, off, :]
            nc.vector.tensor_tensor_scan(
                ecur[:, 1:C + 1], s_t[:], wsl, ecur[:, 0:1],
                op0=ALU.add, op1=ALU.mult,
            )
            if t >= TAIL0:
                nc.vector.tensor_copy(
                    bcol[:, t - TAIL0:t - TAIL0 + 1], ecur[:, C:C + 1]
                )
            if t == EROW_STEP:
                nc.vector.tensor_copy(erow[:], ecur[:])
            # route last col to slot k+1 (partition +8), consumed at t+1
            if t + 2 < NSTEP:
                ps = psum.tile([P, 1], f32, tag="sh")
                nc.tensor.matmul(
                    ps[:], shm[:], ecur[:, C:C + 1], start=True, stop=True
                )
                mm_ps[t] = ps
            mm_ps.pop(t - 3, None)
            e_tiles.pop(t - 3, None)

        # ---- finalization: out = [E_last, Lacc]
        outt = const.tile([P, 2], f32)
        nc.vector.tensor_copy(outt[:, 0:1], e_tiles[NSTEP - 1][:, C:C + 1])
        nc.vector.tensor_copy(outt[:, 1:2], self.lacc_cur[:])
        nc.sync.dma_start(out_d, outt[:])
        nc.sync.dma_start(bcol_d, bcol[:])
        nc.sync.dma_start(erow_d, erow[:])
        nc.sync.dma_start(lh_d, lh[:])

    # ------------------------------------------------------------------ W
    def produce_quarter(self, chunk, q, wt, veng):
        """Produce W[:, 16q:16(q+1), :] of `chunk` into tile wt.
        veng: tensor-op engine namespace (nc.vector or nc.gpsimd)."""
        nc = self.nc_
        f32 = self.f32
        t0 = chunk * RBLK + q * QTR
        nm = f"{chunk}_{q}"
        dxq = self.qpool.tile([P, QTR, C], f32, tag="dx", name="dx" + nm)
        dyq = self.qpool.tile([P, QTR, C], f32, tag="dy", name="dy" + nm)
        sqx = self.qpool.tile([P, QTR, C], f32, tag="sqx", name="sqx" + nm)
        sqy = self.qpool.tile([P, QTR, C], f32, tag="sqy", name="sqy" + nm)
        dq = self.qpool.tile([P, QTR, C], f32, tag="dq", name="dq" + nm)
        cxb = self.cx[:].unsqueeze(1).broadcast_to([P, QTR, C])
        cyb = self.cy[:].unsqueeze(1).broadcast_to([P, QTR, C])
        sxb = self.sx[:, t0:t0 + QTR].unsqueeze(2).broadcast_to([P, QTR, C])
        syb = self.sy[:, t0:t0 + QTR].unsqueeze(2).broadcast_to([P, QTR, C])
        steps = [
            lambda: veng.tensor_sub(dxq[:], cxb, sxb),
            lambda: nc.scalar.activation(sqx[:], dxq[:], AF.Square),
            lambda: veng.tensor_sub(dyq[:], cyb, syb),
            lambda: nc.scalar.activation(sqy[:], dyq[:], AF.Square),
            lambda: veng.tensor_add(dq[:], sqx[:], sqy[:]),
            lambda: nc.scalar.activation(
                wt[:, q * QTR:(q + 1) * QTR, :], dq[:], AF.Exp, scale=-1.0
            ),
        ]
        return steps

    def produce_chunk(self, chunk, veng):
        wt = self.wpool.tile([P, RBLK, C], self.f32, tag="w")
        for q in range(RBLK // QTR):
            for s in self.produce_quarter(chunk, q, wt, veng):
                s()
        return wt

    def produce_slice(self, chunk, off, w_tiles):
        """Emit one pipeline stage of next-chunk production at group `off`.
        Per quarter q (window of 16 groups): sub dx @+1, sub dy @+5,
        add @+9 (Pool); Square/Exp on Act chase their producers."""
        q, ph = off // QTR, off % QTR
        if ph == 1 and q == 0 and chunk not in w_tiles:
            w_tiles[chunk] = self.wpool.tile([P, RBLK, C], self.f32, tag="w", name=f"wch{chunk}")
        if ph == 1:
            key = (chunk, q)
            self.rst[key] = self.produce_quarter(
                chunk, q, w_tiles[chunk], self.nc_.gpsimd
            )
            self.rst[key][0]()  # dx
            self.rst[key][1]()  # sqx (Act)
        elif ph == 5:
            self.rst[(chunk, q)][2]()  # dy
            self.rst[(chunk, q)][3]()  # sqy
        elif ph == 9:
            self.rst[(chunk, q)][4]()  # d = sqx + sqy
            self.rst[(chunk, q)][5]()  # w = exp(-d)
            del self.rst[(chunk, q)]

    # ------------------------------------------------------------ rescale
    def rescale_phase(self, ph, T, e_tiles):
        """Tiny-op scale chain for epoch T, spread over groups T-8..T-1.
        Reads the row produced at step T-DLEAD-1 (stale anchor). All DVE
        ops are [P,1] so they hide in the chain's RAW-semaphore gaps."""
        nc = self.nc_
        f32, u32 = self.f32, self.u32
        st = self.rst.setdefault(("rs", T), {})
        sm = self.small
        X = mybir.AxisListType.X

        def tl(name, dt=f32):
            st[name] = sm.tile([P, 1], dt, tag="rs_" + name, name=f"rs_{name}_{T}")
            return st[name]

        if ph == 0:
            prev = e_tiles[T - DLEAD - 1]
            nc.vector.tensor_reduce(
                tl("m")[:], prev[:], axis=X, op=ALU.max
            )
        elif ph == 1:
            nc.vector.tensor_scalar(
                tl("z")[:], st["m"][:], 1e-37, None, op0=ALU.is_le
            )
            nc.vector.scalar_tensor_tensor(
                tl("mz")[:], st["z"][:], TWO64, st["m"][:],
                op0=ALU.mult, op1=ALU.add,
            )
            nc.vector.tensor_scalar(
                tl("eu", u32)[:], st["mz"][:].bitcast(u32), 23, None,
                op0=ALU.logical_shift_right,
            )
        elif ph == 2:
            nc.vector.tensor_copy(tl("ef")[:], st["eu"][:])   # u32 -> f32
            nc.vector.scalar_tensor_tensor(
                tl("lc")[:], st["ef"][:], HEADROOM, self.lacc_cur[:],
                op0=ALU.add, op1=ALU.add,
            )
            nc.vector.scalar_tensor_tensor(
                tl("lsrc")[:], st["z"][:], -3.0e8, st["lc"][:],
                op0=ALU.mult, op1=ALU.add,
            )
        elif ph == 3:
            st["psl"] = self.psum.tile([P, 1], f32, tag="psl", name=f"psl_{T}")
            nc.tensor.matmul(
                st["psl"][:], self.shm[:], st["lsrc"][:], start=True, stop=True
            )
            nc.vector.tensor_tensor(
                tl("nb")[:], st["psl"][:], self.negmask[:], op=ALU.add
            )
        elif ph == 4:
            nc.vector.tensor_tensor(
                tl("mx")[:], st["lsrc"][:], st["nb"][:], op=ALU.max
            )
            nc.vector.tensor_scalar(
                tl("vv")[:], st["nb"][:], -1e8, None, op0=ALU.is_ge
            )
            nc.vector.tensor_tensor(
                st["mx"][:], st["mx"][:], st["lc"][:], op=ALU.subtract
            )
        elif ph == 5:
            nc.vector.tensor_tensor(
                st["mx"][:], st["mx"][:], st["vv"][:], op=ALU.mult
            )
            lnew = self.lacc_alt
            st["lnew"] = lnew
            nc.vector.tensor_tensor(
                lnew[:], st["lc"][:], st["mx"][:], op=ALU.add
            )
            nc.vector.scalar_tensor_tensor(
                tl("sfe")[:], lnew[:], -1.0, self.lacc_cur[:],
                op0=ALU.mult, op1=ALU.add,
            )
        elif ph == 6:
            nc.vector.tensor_scalar(
                st["sfe"][:], st["sfe"][:], -126.0, 126.0,
                op0=ALU.max, op1=ALU.min,
            )
            nc.vector.tensor_scalar(
                st["sfe"][:], st["sfe"][:], 127.0, None, op0=ALU.add
            )
            nc.vector.tensor_copy(tl("sfu", u32)[:], st["sfe"][:])
            nc.vector.tensor_scalar(
                self.sf_t[:], st["sfu"][:], 23, None,
                op0=ALU.logical_shift_left,
            )
        elif ph == 7:
            st["psl2"] = self.psum.tile([P, 1], f32, tag="psl2", name=f"psl2_{T}")
            nc.tensor.matmul(
                st["psl2"][:], self.shm[:], st["lnew"][:], start=True, stop=True
            )
            nc.vector.tensor_tensor(
                tl("dl")[:], st["psl2"][:], self.negmask[:], op=ALU.add
            )
            nc.vector.tensor_tensor(
                st["dl"][:], st["dl"][:], st["lnew"][:], op=ALU.subtract
            )
        elif ph == 8:  # group T-... wait: ph runs 0..DLEAD-1 then T itself
            pass

        if ph == DLEAD - 1:
            # finish r for the new regime; needed first at group T+1
            nc.vector.tensor_scalar(
                st["dl"][:], st["dl"][:], -126.0, 110.0,
                op0=ALU.max, op1=ALU.min,
            )
            nc.vector.tensor_scalar(
                st["dl"][:], st["dl"][:], 127.0, None, op0=ALU.add
            )
            nc.vector.tensor_copy(tl("ru", u32)[:], st["dl"][:])
            regime = T // Q
            nc.vector.tensor_scalar(
                self.r_tiles[regime % 2][:], st["ru"][:], 23, None,
                op0=ALU.logical_shift_left,
            )
            if T >= TAIL0:
                kk = (T - TAIL0) // Q
                nc.vector.tensor_copy(
                    self.lh[:, kk:kk + 1], st["lnew"][:]
                )
            self.lacc_cur, self.lacc_alt = st["lnew"], self.lacc_cur
            del self.rst[("rs", T)]


def prep_core_inputs(snake, contour):
    """snake, contour: [B, N, 2] float32 -> input dict for one core."""
    cx = np.empty((P, C), np.float32)
    cy = np.empty((P, C), np.float32)
    sx = np.full((P, SXLEN), PAD, np.float32)
    sy = np.full((P, SXLEN), PAD, np.float32)
    for k in range(K):
        for b in range(B):
            p = k * B + b
            cx[p] = contour[b, k * C:(k + 1) * C, 0]
            cy[p] = contour[b, k * C:(k + 1) * C, 1]
            lo = 2 * k
            sx[p, lo:lo + N] = snake[b, :, 0]
            sy[p, lo:lo + N] = snake[b, :, 1]
    shift = np.zeros((P, P), np.float32)
    for q in range(P - B):
        shift[q, q + B] = 1.0
    return {"cx": cx, "cy": cy, "sx": sx, "sy": sy, "shift": shift}


_CACHED = {}


def _get_nc():
    if "nc" not in _CACHED:
        _CACHED["nc"] = build_bass()
    return _CACHED["nc"]


def host_finish(out_map, snake, contour):
    """Recompute the final CxC block in float64 log space from exported
    boundaries (the corner can sit beyond f32 range below the block peak).
    snake, contour: [B, N, 2] for this core. Returns R[B]."""
    LN2 = np.log(2.0)
    bcol = out_map["bcol"].astype(np.float64)   # [P, NTAIL]
    erow = out_map["erow"].astype(np.float64)   # [P, C+1]
    lh = out_map["lh"].astype(np.float64)       # [P, NHIST]
    i0 = N - C
    res = np.empty(B)
    for b in range(B):
        p15 = (K - 1) * B + b
        p14 = (K - 2) * B + b
        with np.errstate(divide="ignore"):
            # R[i0-1, j], j = i0-1 .. N-1 (erow col 0 is j = i0-1)
            sc15 = lh[p15, (EROW_STEP - TAIL0) // Q]
            Rrow = -(np.log(erow[p15]) + LN2 * sc15)
            # R[i, i0-1], i = i0 .. N-1: slot-14 last col at step i + 2(K-2)
            tt = i0 + np.arange(C) + 2 * (K - 2)
            sc = lh[p14, (tt - TAIL0) // Q]
            Rcol = -(np.log(bcol[p14, tt - TAIL0]) + LN2 * sc)
        D = ((snake[b, i0:, None, :].astype(np.float64)
              - contour[b, None, i0:, :].astype(np.float64)) ** 2).sum(-1)
        Rm = np.full((C + 1, C + 1), np.inf)
        Rm[0, :] = Rrow
        Rm[1:, 0] = Rcol
        for ii in range(1, C + 1):
            dvals = D[ii - 1]
            rowm1 = Rm[ii - 1]
            rowc = Rm[ii]
            for jj in range(1, C + 1):
                v0, v1, v2 = rowm1[jj - 1], rowm1[jj], rowc[jj - 1]
                mn = min(v0, v1, v2)
                if mn == np.inf:
                    continue
                rowc[jj] = dvals[jj - 1] + mn - np.log(
                    np.exp(mn - v0) + np.exp(mn - v1) + np.exp(mn - v2)
                )
        res[b] = Rm[C, C]
    return res


def run(snake, contour, trace=False):
    """Returns (loss, results_obj)."""
    snake = np.asarray(snake, np.float32)
    contour = np.asarray(contour, np.float32)
    nbatch = snake.shape[0]
    assert nbatch == NCORES * B, (snake.shape, contour.shape)
    in_maps = [
        prep_core_inputs(
            snake[c * B:(c + 1) * B], contour[c * B:(c + 1) * B]
        )
        for c in range(NCORES)
    ]
    nc = _get_nc()
    res = bass_utils.run_bass_kernel_spmd(
        nc, in_maps, core_ids=list(range(NCORES)), trace=trace
    )
    rs = []
    for c in range(NCORES):
        rs.append(host_finish(
            res.results[c],
            snake[c * B:(c + 1) * B], contour[c * B:(c + 1) * B],
        ))
    loss = np.mean(np.concatenate(rs), dtype=np.float64)
    return np.float32(loss), res


def kernel(snake, contour):
    loss, _ = run(snake, contour, trace=False)
    return np.array(loss, dtype=np.float32)


# revision 16
# speedup vs baseline: 2.1557x; 1.0105x over previous
"""SoftDTW loss (AbstractDTW, gamma=1) Trainium2 Bass kernel.

Algorithm: exp-space linearization of the SoftDTW DP. With E = exp(-R) and
W = exp(-D), the recurrence R[i,j] = D[i,j] + softmin(R[i-1,j-1], R[i-1,j],
R[i,j-1]) becomes linear:

    E[i,j] = W[i,j] * (E[i-1,j-1] + E[i-1,j] + E[i,j-1])

Per row this is a first-order linear scan along j, which maps onto the DVE
`tensor_tensor_scan` instruction with op0=add, op1=mult:

    state = (S[j] + state) * W[j],   S[j] = E[i-1,j-1] + E[i-1,j]

Parallelization per core: 8 batch items; the 1024 columns are split into
K=16 blocks of C=64. SBUF partition p = k*8 + b (128 partitions). Wavefront
skew: slot k processes row i at step t = i + 2k, so cross-slot boundary
values (last column of slot k-1) arrive 2 steps ahead of use.

Engine split (the DVE add->scan chain is the critical path; every DVE op
larger than the ~95ns RAW-semaphore gap pushes the whole schedule):
  - DVE: per-step pair-sum + scan + boundary import (PSUM->SBUF with the
    power-of-two scale ratio r, hidden in the RAW gap), plus the rescale
    chain as tiny [P,1] ops spread into chain gaps.
  - Pool (gpsimd): all W-production tensor passes (sub, sub, add) in
    [P,16,C] slices. Nothing latency-critical lives here.
  - Act: W-production activations (Square x2, Exp).
  - PE: boundary route (shift-by-8 matmul) + 2 rescale routes per epoch.

Numerical range: per-slot scales, every factor an exact power of 2 built by
exponent-field bit arithmetic (HW Ln/Exp LUTs are unsafe in the far tails).
Scales re-anchor every Q=16 steps; the anchor row is sampled DLEAD=8 steps
early so the whole scale computation runs off the critical chain, and only
one tensor_scalar_mul (plus two [P,1] boundary fixups) lands on it per
epoch. The final CxC block is recomputed on host in float64 log space from
exported boundary values. Batch dim (64) sharded 8 ways; mean on host.
"""

import sys
from contextlib import ExitStack

import numpy as np

sys.path.insert(0, "/opt/trn_rl_repo")

import concourse.bass as bass  # noqa: E402
import concourse.tile as tile  # noqa: E402
from concourse import bacc, mybir  # noqa: E402
from concourse import bass_utils  # noqa: E402

AF = mybir.ActivationFunctionType
ALU = mybir.AluOpType

NCORES = 8
B = 8          # batch per core
K = 16         # column slots
N = 1024       # sequence length
C = N // K     # columns per slot
P = K * B      # 128 partitions
NSTEP = N + 2 * (K - 1)          # 1054 wavefront steps
RBLK = 64                        # W-production chunk (steps)
QTR = 16                         # production slice (steps)
NCHUNK = (NSTEP + RBLK - 1) // RBLK
SXLEN = NCHUNK * RBLK            # staggered snake buffer length (1088)
Q = 16                           # rescale cadence
DLEAD = 12                       # rescale anchor staleness (steps)
RESCALE_STEPS = [t for t in range(Q, NSTEP) if t % Q == 0]
PAD = np.float32(1e4)            # out-of-range snake pad -> W == 0
TAIL0 = ((N - C - 1 + 2 * (K - 2)) // Q) * Q   # export window start step
NTAIL = NSTEP - TAIL0                          # boundary-column exports
EROW_STEP = N - C - 1 + 2 * (K - 1)            # step producing row N-C-1
NHIST = (NSTEP - 1 - TAIL0) // Q + 1           # lacc snapshots in window
HEADROOM = -191.0                # -(127 + 64): anchor 2^64 below block peak
# filler with exponent field == -HEADROOM so empty slots keep lcand == lacc
FILLER = np.float32(2.0 ** (-HEADROOM - 127.0))


def build_bass():
    """Build the per-core Bass program (SPMD: same program on all cores)."""
    nc = bacc.Bacc(
        "TRN2",
        target_bir_lowering=False,
        debug=False,
        enable_asserts=False,
        num_devices=NCORES,
    )
    f32 = mybir.dt.float32
    cx_d = nc.dram_tensor("cx", [P, C], f32, kind="ExternalInput").ap()
    cy_d = nc.dram_tensor("cy", [P, C], f32, kind="ExternalInput").ap()
    sx_d = nc.dram_tensor("sx", [P, SXLEN], f32, kind="ExternalInput").ap()
    sy_d = nc.dram_tensor("sy", [P, SXLEN], f32, kind="ExternalInput").ap()
    sh_d = nc.dram_tensor("shift", [P, P], f32, kind="ExternalInput").ap()
    out_d = nc.dram_tensor("out", [P, 2], f32, kind="ExternalOutput").ap()
    bcol_d = nc.dram_tensor("bcol", [P, NTAIL], f32, kind="ExternalOutput").ap()
    erow_d = nc.dram_tensor("erow", [P, C + 1], f32, kind="ExternalOutput").ap()
    lh_d = nc.dram_tensor("lh", [P, NHIST], f32, kind="ExternalOutput").ap()

    with TileKernel(nc) as tk:
        tk.body(cx_d, cy_d, sx_d, sy_d, sh_d, out_d, bcol_d, erow_d, lh_d)
    nc.compile()
    return nc


class TileKernel:
    def __init__(self, nc):
        self.nc = nc
        self.ctx = ExitStack()
        self.tc = tile.TileContext(nc)

    def __enter__(self):
        self.ctx.__enter__()
        self.tc.__enter__()
        return self

    def __exit__(self, *a):
        self.ctx.__exit__(*a)  # close tile pools before scheduling
        return self.tc.__exit__(*a)

    def body(self, cx_d, cy_d, sx_d, sy_d, sh_d, out_d, bcol_d, erow_d, lh_d):
        nc = self.nc
        tc = self.tc
        ctx = self.ctx
        f32 = mybir.dt.float32
        u32 = mybir.dt.uint32

        const = ctx.enter_context(tc.tile_pool(name="const", bufs=1))
        wpool = ctx.enter_context(tc.tile_pool(name="wp", bufs=2))
        qpool = ctx.enter_context(tc.tile_pool(name="qp", bufs=2))
        epool = ctx.enter_context(tc.tile_pool(name="ep", bufs=6))
        spool = ctx.enter_context(tc.tile_pool(name="sp", bufs=3))
        small = ctx.enter_context(tc.tile_pool(name="sm", bufs=2))
        psum = ctx.enter_context(tc.tile_pool(name="ps", bufs=2, space="PSUM"))

        # ---- constants / inputs
        cx = const.tile([P, C], f32)
        cy = const.tile([P, C], f32)
        sx = const.tile([P, SXLEN], f32)
        sy = const.tile([P, SXLEN], f32)
        shm = const.tile([P, P], f32)
        nc.sync.dma_start(cx[:], cx_d)
        nc.sync.dma_start(cy[:], cy_d)
        nc.sync.dma_start(sx[:], sx_d)
        nc.sync.dma_start(sy[:], sy_d)
        nc.sync.dma_start(shm[:], sh_d)

        # accumulated base-2 exponent per partition (exact f32 integers)
        lacc_a = const.tile([P, 1], f32)
        lacc_b = const.tile([P, 1], f32)
        nc.vector.memset(lacc_a[:], 0.0)
        self.lacc_cur, self.lacc_alt = lacc_a, lacc_b
        # r per scale regime: 2^(L[p-8]-L[p]) as bitcast-f32 (power of 2)
        r_a = const.tile([P, 1], u32)
        r_b = const.tile([P, 1], u32)
        nc.vector.memset(r_a[:].bitcast(f32), 1.0)
        self.r_tiles = [r_a, r_b]
        # per-epoch value scale factor 2^(Lacc-Lnew) bits
        self.sf_t = const.tile([P, 1], u32)
        # additive mask: slot 0 rows get a huge negative so max() keeps own
        negmask = const.tile([P, 1], f32)
        nc.vector.memset(negmask[:], 0.0)
        nc.vector.memset(negmask[0:B, :], -3.0e8)

        bcol = const.tile([P, NTAIL], f32)
        erow = const.tile([P, C + 1], f32)
        lh = const.tile([P, NHIST], f32)

        self.nc_ = nc
        self.cx, self.cy, self.sx, self.sy = cx, cy, sx, sy
        self.shm, self.negmask = shm, negmask
        self.qpool, self.wpool, self.psum, self.small = qpool, wpool, psum, small
        self.lh, self.f32, self.u32 = lh, f32, u32

        # ---- prologue: produce W chunk 0 quarters 0-1 on DVE+Act so the
        # chain starts early; quarters 2-3 go to Pool at loop start (they
        # complete well before steps 32/48 need them)
        w_tiles = {}
        w_tiles[0] = self.produce_chunk(0, nc.vector, quarters=(0, 1))

        # E tile ring; e[t] has C+1 cols: col 0 = boundary, 1..C = block
        e_tiles = {}
        e_init = epool.tile([P, C + 1], f32)
        nc.vector.memset(e_init[:], 0.0)
        nc.vector.memset(e_init[0:B, 0:1], 1.0)   # DP seed delta (slot 0)
        e_tiles[-1] = e_init

        self.rst = {}  # live rescale-chain state
        mm_ps = {}     # step -> psum tile of its boundary route

        for t in range(NSTEP):
            cchunk = t // RBLK
            off = t % RBLK
            if t == 0:
                for q in (2, 3):
                    for s in self.produce_quarter(0, q, w_tiles[0], nc.gpsimd):
                        s()
            # ---- W production for chunk c+1 on Pool/Act, sliced
            if cchunk + 1 < NCHUNK:
                self.produce_slice(cchunk + 1, off, w_tiles)

            # ---- rescale chain (tiny DVE ops in chain gaps + PE routes)
            Tnext = ((t + DLEAD) // Q) * Q
            if Tnext in RESCALE_STEPS and t >= Tnext - DLEAD:
                self.rescale_phase(t - (Tnext - DLEAD), Tnext, e_tiles)

            # ---- wavefront step t
            is_epoch = t in RESCALE_STEPS
            eprev = e_tiles[t - 1]
            if t not in e_tiles:  # t = 0, 1: no boundary was routed yet
                ecur = epool.tile([P, C + 1], f32)
                nc.vector.memset(ecur[:, 0:1], 0.0)
                e_tiles[t] = ecur
            ecur = e_tiles[t]

            def bimport():
                # boundary import for step t+1 (fills the add->scan RAW gap)
                if t >= 1 and t + 1 < NSTEP:
                    regime = (t - 1) // Q
                    r_ap = self.r_tiles[regime % 2][:].bitcast(f32)
                    enx = epool.tile(
                        [P, C + 1], f32, tag="enx", name=f"e{t + 1}"
                    )
                    e_tiles[t + 1] = enx
                    nc.vector.tensor_scalar_mul(
                        enx[:, 0:1], mm_ps[t - 1][:], r_ap
                    )

            s_t = spool.tile([P, C], f32)
            if is_epoch:
                sf_ap = self.sf_t[:].bitcast(f32)
                nc.vector.tensor_scalar_mul(eprev[:], eprev[:], sf_ap)
                nc.vector.tensor_add(s_t[:], eprev[:, 0:C], eprev[:, 1:C + 1])
                bimport()
                nc.vector.tensor_scalar_mul(ecur[:, 0:1], ecur[:, 0:1], sf_ap)
                if t + 1 in e_tiles:
                    nc.vector.tensor_scalar_mul(
                        e_tiles[t + 1][:, 0:1], e_tiles[t + 1][:, 0:1], sf_ap
                    )
            else:
                nc.vector.tensor_add(s_t[:], eprev[:, 0:C], eprev[:, 1:C + 1])
                bimport()
            wsl = w_tiles[cchunk][:, off, :]
            nc.vector.tensor_tensor_scan(
                ecur[:, 1:C + 1], s_t[:], wsl, ecur[:, 0:1],
                op0=ALU.add, op1=ALU.mult,
            )
            if t >= TAIL0:
                nc.vector.tensor_copy(
                    bcol[:, t - TAIL0:t - TAIL0 + 1], ecur[:, C:C + 1]
                )
            if t == EROW_STEP:
                nc.vector.tensor_copy(erow[:], ecur[:])
                nc.sync.dma_start(erow_d, erow[:])
            if t == 1041:
                # lh fully written by the T=1040 epoch; overlap its export
                nc.sync.dma_start(lh_d, lh[:])
                nc.sync.dma_start(bcol_d[:, 0:64], bcol[:, 0:64])
            # route last col to slot k+1 (partition +8), consumed at t+1
            if t + 2 < NSTEP:
                ps = psum.tile([P, 1], f32, tag="sh")
                nc.tensor.matmul(
                    ps[:], shm[:], ecur[:, C:C + 1], start=True, stop=True
                )
                mm_ps[t] = ps
            mm_ps.pop(t - 3, None)
            e_tiles.pop(t - 3, None)

        # ---- finalization: out = [E_last, Lacc]
        outt = const.tile([P, 2], f32)
        nc.vector.tensor_copy(outt[:, 0:1], e_tiles[NSTEP - 1][:, C:C + 1])
        nc.vector.tensor_copy(outt[:, 1:2], self.lacc_cur[:])
        nc.sync.dma_start(out_d, outt[:])
        nc.sync.dma_start(bcol_d[:, 64:NTAIL], bcol[:, 64:NTAIL])

    # ------------------------------------------------------------------ W
    def produce_quarter(self, chunk, q, wt, veng):
        """Produce W[:, 16q:16(q+1), :] of `chunk` into tile wt.
        veng: tensor-op engine namespace (nc.vector or nc.gpsimd)."""
        nc = self.nc_
        f32 = self.f32
        t0 = chunk * RBLK + q * QTR
        nm = f"{chunk}_{q}"
        dxq = self.qpool.tile([P, QTR, C], f32, tag="dx", name="dx" + nm)
        dyq = self.qpool.tile([P, QTR, C], f32, tag="dy", name="dy" + nm)
        sqx = self.qpool.tile([P, QTR, C], f32, tag="sqx", name="sqx" + nm)
        sqy = self.qpool.tile([P, QTR, C], f32, tag="sqy", name="sqy" + nm)
        dq = self.qpool.tile([P, QTR, C], f32, tag="dq", name="dq" + nm)
        cxb = self.cx[:].unsqueeze(1).broadcast_to([P, QTR, C])
        cyb = self.cy[:].unsqueeze(1).broadcast_to([P, QTR, C])
        sxb = self.sx[:, t0:t0 + QTR].unsqueeze(2).broadcast_to([P, QTR, C])
        syb = self.sy[:, t0:t0 + QTR].unsqueeze(2).broadcast_to([P, QTR, C])
        steps = [
            lambda: veng.tensor_sub(dxq[:], cxb, sxb),
            lambda: nc.scalar.activation(sqx[:], dxq[:], AF.Square),
            lambda: veng.tensor_sub(dyq[:], cyb, syb),
            lambda: nc.scalar.activation(sqy[:], dyq[:], AF.Square),
            lambda: veng.tensor_add(dq[:], sqx[:], sqy[:]),
            lambda: nc.scalar.activation(
                wt[:, q * QTR:(q + 1) * QTR, :], dq[:], AF.Exp, scale=-1.0
            ),
        ]
        return steps

    def produce_chunk(self, chunk, veng, quarters=None):
        wt = self.wpool.tile([P, RBLK, C], self.f32, tag="w")
        if quarters is None:
            quarters = range(RBLK // QTR)
        for q in quarters:
            for s in self.produce_quarter(chunk, q, wt, veng):
                s()
        return wt

    def produce_slice(self, chunk, off, w_tiles):
        """Emit one pipeline stage of next-chunk production at group `off`.
        Per quarter q (window of 16 groups): sub dx @+1, sub dy @+5,
        add @+9 (Pool); Square/Exp on Act chase their producers."""
        q, ph = off // QTR, off % QTR
        if ph == 1 and q == 0 and chunk not in w_tiles:
            w_tiles[chunk] = self.wpool.tile([P, RBLK, C], self.f32, tag="w", name=f"wch{chunk}")
        if ph == 1:
            key = (chunk, q)
            self.rst[key] = self.produce_quarter(
                chunk, q, w_tiles[chunk], self.nc_.gpsimd
            )
            self.rst[key][0]()  # dx
            self.rst[key][1]()  # sqx (Act)
        elif ph == 5:
            self.rst[(chunk, q)][2]()  # dy
            self.rst[(chunk, q)][3]()  # sqy
        elif ph == 9:
            self.rst[(chunk, q)][4]()  # d = sqx + sqy
            self.rst[(chunk, q)][5]()  # w = exp(-d)
            del self.rst[(chunk, q)]

    # ------------------------------------------------------------ rescale
    def rescale_phase(self, ph, T, e_tiles):
        """Tiny-op scale chain for epoch T, spread over groups T-8..T-1.
        Reads the row produced at step T-DLEAD-1 (stale anchor). All DVE
        ops are [P,1] so they hide in the chain's RAW-semaphore gaps."""
        nc = self.nc_
        f32, u32 = self.f32, self.u32
        st = self.rst.setdefault(("rs", T), {})
        sm = self.small
        X = mybir.AxisListType.X

        def tl(name, dt=f32):
            st[name] = sm.tile([P, 1], dt, tag="rs_" + name, name=f"rs_{name}_{T}")
            return st[name]

        if ph == 0:
            prev = e_tiles[T - DLEAD - 1]
            nc.vector.tensor_reduce(
                tl("m")[:], prev[:], axis=X, op=ALU.max
            )
        elif ph == 1:
            nc.vector.tensor_scalar(
                tl("z")[:], st["m"][:], 1e-37, None, op0=ALU.is_le
            )
            nc.vector.scalar_tensor_tensor(
                tl("mz")[:], st["z"][:], float(FILLER), st["m"][:],
                op0=ALU.mult, op1=ALU.add,
            )
            nc.vector.tensor_scalar(
                tl("eu", u32)[:], st["mz"][:].bitcast(u32), 23, None,
                op0=ALU.logical_shift_right,
            )
        elif ph == 2:
            nc.vector.tensor_copy(tl("ef")[:], st["eu"][:])   # u32 -> f32
            nc.vector.scalar_tensor_tensor(
                tl("lc")[:], st["ef"][:], HEADROOM, self.lacc_cur[:],
                op0=ALU.add, op1=ALU.add,
            )
            nc.vector.scalar_tensor_tensor(
                tl("lsrc")[:], st["z"][:], -3.0e8, st["lc"][:],
                op0=ALU.mult, op1=ALU.add,
            )
        elif ph == 3:
            st["psl"] = self.psum.tile([P, 1], f32, tag="psl", name=f"psl_{T}")
            nc.tensor.matmul(
                st["psl"][:], self.shm[:], st["lsrc"][:], start=True, stop=True
            )
            nc.vector.tensor_tensor(
                tl("nb")[:], st["psl"][:], self.negmask[:], op=ALU.add
            )
        elif ph == 4:
            nc.vector.tensor_tensor(
                tl("mx")[:], st["lsrc"][:], st["nb"][:], op=ALU.max
            )
            nc.vector.tensor_scalar(
                tl("vv")[:], st["nb"][:], -1e8, None, op0=ALU.is_ge
            )
            nc.vector.tensor_tensor(
                st["mx"][:], st["mx"][:], st["lc"][:], op=ALU.subtract
            )
        elif ph == 5:
            nc.vector.tensor_tensor(
                st["mx"][:], st["mx"][:], st["vv"][:], op=ALU.mult
            )
            lnew = self.lacc_alt
            st["lnew"] = lnew
            nc.vector.tensor_tensor(
                lnew[:], st["lc"][:], st["mx"][:], op=ALU.add
            )
            nc.vector.scalar_tensor_tensor(
                tl("sfe")[:], lnew[:], -1.0, self.lacc_cur[:],
                op0=ALU.mult, op1=ALU.add,
            )
        elif ph == 6:
            nc.vector.tensor_scalar(
                st["sfe"][:], st["sfe"][:], -126.0, 126.0,
                op0=ALU.max, op1=ALU.min,
            )
            nc.vector.tensor_scalar(
                st["sfe"][:], st["sfe"][:], 127.0, None, op0=ALU.add
            )
            nc.vector.tensor_copy(tl("sfu", u32)[:], st["sfe"][:])
            nc.vector.tensor_scalar(
                self.sf_t[:], st["sfu"][:], 23, None,
                op0=ALU.logical_shift_left,
            )
        elif ph == 7:
            st["psl2"] = self.psum.tile([P, 1], f32, tag="psl2", name=f"psl2_{T}")
            nc.tensor.matmul(
                st["psl2"][:], self.shm[:], st["lnew"][:], start=True, stop=True
            )
            nc.vector.tensor_tensor(
                tl("dl")[:], st["psl2"][:], self.negmask[:], op=ALU.add
            )
            nc.vector.tensor_tensor(
                st["dl"][:], st["dl"][:], st["lnew"][:], op=ALU.subtract
            )

        if ph == DLEAD - 1:
            # finish r for the new regime; needed first at group T+1
            nc.vector.tensor_scalar(
                st["dl"][:], st["dl"][:], -126.0, 110.0,
                op0=ALU.max, op1=ALU.min,
            )
            nc.vector.tensor_scalar(
                st["dl"][:], st["dl"][:], 127.0, None, op0=ALU.add
            )
            nc.vector.tensor_copy(tl("ru", u32)[:], st["dl"][:])
            regime = T // Q
            nc.vector.tensor_scalar(
                self.r_tiles[regime % 2][:], st["ru"][:], 23, None,
                op0=ALU.logical_shift_left,
            )
            if T >= TAIL0:
                kk = (T - TAIL0) // Q
                nc.vector.tensor_copy(
                    self.lh[:, kk:kk + 1], st["lnew"][:]
                )
            self.lacc_cur, self.lacc_alt = st["lnew"], self.lacc_cur
            del self.rst[("rs", T)]


def prep_core_inputs(snake, contour):
    """snake, contour: [B, N, 2] float32 -> input dict for one core."""
    cx = np.empty((P, C), np.float32)
    cy = np.empty((P, C), np.float32)
    sx = np.full((P, SXLEN), PAD, np.float32)
    sy = np.full((P, SXLEN), PAD, np.float32)
    for k in range(K):
        for b in range(B):
            p = k * B + b
            cx[p] = contour[b, k * C:(k + 1) * C, 0]
            cy[p] = contour[b, k * C:(k + 1) * C, 1]
            lo = 2 * k
            sx[p, lo:lo + N] = snake[b, :, 0]
            sy[p, lo:lo + N] = snake[b, :, 1]
    shift = np.zeros((P, P), np.float32)
    for q in range(P - B):
        shift[q, q + B] = 1.0
    return {"cx": cx, "cy": cy, "sx": sx, "sy": sy, "shift": shift}


_CACHED = {}


def _get_nc():
    if "nc" not in _CACHED:
        _CACHED["nc"] = build_bass()
    return _CACHED["nc"]


def host_finish(out_map, snake, contour):
    """Recompute the final CxC block in float64 log space from exported
    boundaries (the corner can sit beyond f32 range below the block peak).
    snake, contour: [B, N, 2] for this core. Returns R[B]."""
    LN2 = np.log(2.0)
    bcol = out_map["bcol"].astype(np.float64)   # [P, NTAIL]
    erow = out_map["erow"].astype(np.float64)   # [P, C+1]
    lh = out_map["lh"].astype(np.float64)       # [P, NHIST]
    i0 = N - C
    res = np.empty(B)
    for b in range(B):
        p15 = (K - 1) * B + b
        p14 = (K - 2) * B + b
        with np.errstate(divide="ignore"):
            # R[i0-1, j], j = i0-1 .. N-1 (erow col 0 is j = i0-1)
            sc15 = lh[p15, (EROW_STEP - TAIL0) // Q]
            Rrow = -(np.log(erow[p15]) + LN2 * sc15)
            # R[i, i0-1], i = i0 .. N-1: slot-14 last col at step i + 2(K-2)
            tt = i0 + np.arange(C) + 2 * (K - 2)
            sc = lh[p14, (tt - TAIL0) // Q]
            Rcol = -(np.log(bcol[p14, tt - TAIL0]) + LN2 * sc)
        D = ((snake[b, i0:, None, :].astype(np.float64)
              - contour[b, None, i0:, :].astype(np.float64)) ** 2).sum(-1)
        Rm = np.full((C + 1, C + 1), np.inf)
        Rm[0, :] = Rrow
        Rm[1:, 0] = Rcol
        for ii in range(1, C + 1):
            dvals = D[ii - 1]
            rowm1 = Rm[ii - 1]
            rowc = Rm[ii]
            for jj in range(1, C + 1):
                v0, v1, v2 = rowm1[jj - 1], rowm1[jj], rowc[jj - 1]
                mn = min(v0, v1, v2)
                if mn == np.inf:
                    continue
                rowc[jj] = dvals[jj - 1] + mn - np.log(
                    np.exp(mn - v0) + np.exp(mn - v1) + np.exp(mn - v2)
                )
        res[b] = Rm[C, C]
    return res


def run(snake, contour, trace=False):
    """Returns (loss, results_obj)."""
    snake = np.asarray(snake, np.float32)
    contour = np.asarray(contour, np.float32)
    nbatch = snake.shape[0]
    assert nbatch == NCORES * B, (snake.shape, contour.shape)
    in_maps = [
        prep_core_inputs(
            snake[c * B:(c + 1) * B], contour[c * B:(c + 1) * B]
        )
        for c in range(NCORES)
    ]
    nc = _get_nc()
    res = bass_utils.run_bass_kernel_spmd(
        nc, in_maps, core_ids=list(range(NCORES)), trace=trace
    )
    rs = []
    for c in range(NCORES):
        rs.append(host_finish(
            res.results[c],
            snake[c * B:(c + 1) * B], contour[c * B:(c + 1) * B],
        ))
    loss = np.mean(np.concatenate(rs), dtype=np.float64)
    return np.float32(loss), res


def kernel(snake, contour):
    loss, _ = run(snake, contour, trace=False)
    return np.array(loss, dtype=np.float32)
